# revision 27
# baseline (speedup 1.0000x reference)
"""Trainium2 Bass kernel for nn_Atten2Map (DeePMD dpa2 Atten2Map-style sparse attention).

Contract: kernel(**inputs) takes FULL unsharded numpy inputs
(g2 [2,512,128,64], h2 [2,512,128,3], nlist_mask [2,512,128] bool,
sw [2,512,128], Wqk [64,512]) and returns the full output
[2,512,128,128,4] float32. Internally shards the nb*nloc=1024 atoms
data-parallel across 8 NeuronCores.

Math per atom (nnei=128 neighbors, ND=64, NH=4 heads):
  qk   = g2 @ Wqk                  -> q_h, k_h     [128, 64] each
  raw  = q_h @ k_h^T / sqrt(64)    (scores)
  hh   = h2 @ h2^T                 (gate)
  t    = (raw * hh + 20) * sw_i * sw_j - 20
  a    = softmax(t, axis=-1)  (the -20 constant shift cancels in softmax)
  outh = a * mask_i * mask_j * sw_i * sw_j * hh / sqrt(3)
  out[i, j, h] = outh[i, j]

Device formulation:
  W2_h   = Wq_h @ Wk_h^T / 8   (host, 64x64; scores = G @ W2_h @ G^T)
  tmpT_h = W2_h^T @ G^T        (PE)           [64(e), 128(i)]
  X_h    = tmpT_h^T @ G^T      (PE, scores)   [128(i), 128(j)]
  V1     = (X * sw_i) * hh     (DVE scalar_tensor_tensor, reads PSUM)
  V2     = (V1 + 20*sw_i) * swj_bcast          (DVE STT; swj_bcast from a
            K=1 PE outer product ones^T @ sw_row, read from PSUM)
  E_h, rowsum_h = exp(V2_h - 60) with fused accumulation (ACT)
  rinv'  = (1/rowsum) * mask_i * sw_i / sqrt(3)   (DVE, [128,4])
  out_h  = (E_h * rinv'_h) * hhm    (GPSIMD STT, strided write -> [i, j*4+h])
  where hhm = h2 @ (h2 * mask*sw)^T (PE) folds hh * mask_j * sw_j.
"""

import numpy as np
from contextlib import ExitStack

import concourse.bass as bass
import concourse.tile as tile
from concourse import bacc, mybir
from concourse.bass_utils import run_bass_kernel_spmd

ND, NH, SHIFT = 64, 4, 20.0
NNEI, DIN = 128, 64
NCORES = 8
EXPB = 60.0  # constant shift inside exp; cancels in softmax normalization

F32 = mybir.dt.float32
MMDT = mybir.dt.float32r  # fast fp32 matmul mode (1 cyc/row at N>=256)

P = NNEI  # 128


def _r3(ap):
    """[128, n*128] AP viewed as [128, n, 128]."""
    n = ap.shape[1] // P
    return ap.rearrange("p (h j) -> p h j", h=n)


def build_nc(A: int, mmdt=MMDT):
    """Build the per-core Bass program for A atoms (A even)."""
    assert A % 2 == 0
    nc = bacc.Bacc("TRN2", target_bir_lowering=False, debug=False, num_devices=NCORES)
    dp = nc.declare_dram_parameter
    # pack layout (columns): [g2T: A*DIN][swiT: A][swi20T: A][rmT: A][ident: P][w2p: 2*P (rows 0:64)]
    PACKW = A * DIN + 3 * A + P + 2 * P
    pack = dp("pack", [P, PACKW], MMDT, isOutput=False)
    h2T = dp("h2T", [A, 3, P], MMDT, isOutput=False)
    h2swT = dp("h2swT", [A, 3, P], MMDT, isOutput=False)
    h2mT = dp("h2mT", [A, 3, P], MMDT, isOutput=False)
    swrow = dp("swrow", [1, A * P], F32, isOutput=False)
    out = dp("out", [A, P, P * NH], F32, isOutput=True)

    mm = lambda ap: ap.bitcast(mmdt)
    AF = mybir.ActivationFunctionType
    OP = mybir.AluOpType

    with tile.TileContext(nc) as tc, ExitStack() as ctx:
        sb = ctx.enter_context(tc.tile_pool(name="persist", bufs=1))
        # one packed persistent load: a single DMA -> a single wait semaphore
        pack_s = sb.tile([P, PACKW], MMDT)
        nc.gpsimd.dma_start(pack_s[:, :], pack[:, :])
        o = A * DIN
        g_s = pack_s[:, 0:o]
        swiT_s = pack_s[:, o:o + A].bitcast(F32); o += A
        swi20T_s = pack_s[:, o:o + A].bitcast(F32); o += A
        rmT_s = pack_s[:, o:o + A].bitcast(F32); o += A
        id_s = pack_s[:, o:o + P]; o += P
        w2p_s = pack_s[0:DIN, o:o + 2 * P]; o += 2 * P
        negb = sb.tile([P, 1], F32)
        nc.vector.memset(negb[:, :], -EXPB)

        # pools
        ht_pool = ctx.enter_context(tc.tile_pool(name="ht", bufs=3))
        gt_pool = ctx.enter_context(tc.tile_pool(name="gt", bufs=3))
        tts_pool = ctx.enter_context(tc.tile_pool(name="tts", bufs=2))
        hh_pool = ctx.enter_context(tc.tile_pool(name="hh", bufs=6))
        work_pool = ctx.enter_context(tc.tile_pool(name="work", bufs=3))
        stat_pool = ctx.enter_context(tc.tile_pool(name="stat", bufs=6))
        # PSUM pools
        pt_pool = ctx.enter_context(tc.tile_pool(name="pt", bufs=1, space="PSUM"))
        ptm_pool = ctx.enter_context(tc.tile_pool(name="ptm", bufs=1, space="PSUM"))
        psc_pool = ctx.enter_context(tc.tile_pool(name="psc", bufs=2, space="PSUM"))
        pmisc_pool = ctx.enter_context(tc.tile_pool(name="pmisc", bufs=2, space="PSUM"))
        swj_pool = ctx.enter_context(tc.tile_pool(name="swj", bufs=2))

        for p in range(A // 2):
            a0, a1 = 2 * p, 2 * p + 1
            # --- H^T tiles (pair): [3, 256]
            ht = ht_pool.tile([3, 2 * P], MMDT, tag="ht")
            nc.gpsimd.dma_start(_r3(ht[:, :]), h2T[a0:a0 + 2, :, :].transpose([1, 0, 2]))
            htsw = ht_pool.tile([3, 2 * P], MMDT, tag="htsw")
            nc.gpsimd.dma_start(_r3(htsw[:, :]), h2swT[a0:a0 + 2, :, :].transpose([1, 0, 2]))
            htm = ht_pool.tile([3, 2 * P], MMDT, tag="htm")
            nc.gpsimd.dma_start(_r3(htm[:, :]), h2mT[a0:a0 + 2, :, :].transpose([1, 0, 2]))

            # --- G transposes -> PSUM [64, 256] -> SBUF
            gt_ps = pt_pool.tile([DIN, 2 * P], MMDT)
            nc.tensor.transpose(gt_ps[:, 0:P], g_s[:, a0 * DIN:(a0 + 1) * DIN], id_s)
            nc.tensor.transpose(gt_ps[:, P:], g_s[:, a1 * DIN:(a1 + 1) * DIN], id_s)
            gts = gt_pool.tile([DIN, 2 * P], MMDT)
            nc.scalar.copy(gts[:, :], gt_ps[:, :])

            # --- tmpT matmuls: per head [64, 256] at base partition 0 -> SBUF [64, 1024]
            tts = tts_pool.tile([DIN, NH * 2 * P], MMDT)
            for hp in range(2):
                ptm = ptm_pool.tile([DIN, 4 * P], F32)
                for hi in range(2):
                    h = 2 * hp + hi
                    nc.tensor.matmul(ptm[:, hi * 2 * P:(hi + 1) * 2 * P],
                                     w2p_s[:, h * ND:(h + 1) * ND], gts[:, :],
                                     start=True, stop=True)
                nc.scalar.copy(tts[:, hp * 4 * P:(hp + 1) * 4 * P], ptm[:, :])

            # --- hh / hhm pair matmuls (half the columns are cross-atom garbage)
            phh = pmisc_pool.tile([P, 4 * P], F32, tag="pmisc")
            nc.tensor.matmul(phh[:, 0:2 * P], ht[:, 0:P], htsw[:, :], start=True, stop=True)
            nc.tensor.matmul(phh[:, 2 * P:], ht[:, P:], htsw[:, :], start=True, stop=True)
            hh0 = hh_pool.tile([P, P], F32, tag="hh")
            nc.scalar.copy(hh0[:, :], phh[:, 0:P])
            hh1 = hh_pool.tile([P, P], F32, tag="hh")
            nc.scalar.copy(hh1[:, :], phh[:, 3 * P:])
            phm = pmisc_pool.tile([P, 4 * P], F32, tag="pmisc")
            nc.tensor.matmul(phm[:, 0:2 * P], ht[:, 0:P], htm[:, :], start=True, stop=True)
            nc.tensor.matmul(phm[:, 2 * P:], ht[:, P:], htm[:, :], start=True, stop=True)
            hm0 = hh_pool.tile([P, P], F32, tag="hm")
            nc.scalar.copy(hm0[:, :], phm[:, 0:P])
            hm1 = hh_pool.tile([P, P], F32, tag="hm")
            nc.scalar.copy(hm1[:, :], phm[:, 3 * P:])

            # --- sw_j broadcast rows (exact fp32): DMA from DRAM, partition-broadcast source
            swjb = swj_pool.tile([P, 2 * P], F32)
            nc.gpsimd.dma_start(swjb[:, :],
                                swrow[0:1, a0 * P:(a0 + 2) * P].broadcast_to([P, 2 * P]))

            for ai, a in ((0, a0), (1, a1)):
                hh_a = (hh0, hh1)[ai]
                hm_a = (hm0, hm1)[ai]
                # --- scores: 2 head-pair PSUM tiles [128, 512] each
                v1 = work_pool.tile([P, 4 * P], F32, tag="v1")
                for hp in range(2):
                    psc = psc_pool.tile([P, 4 * P], F32)
                    for hi in range(2):
                        h = 2 * hp + hi
                        c0 = h * 2 * P + ai * P
                        nc.tensor.matmul(psc[:, hi * 2 * P:(hi + 1) * 2 * P],
                                         tts[:, c0:c0 + P], gts[:, :],
                                         start=True, stop=True)
                    # V1 = (X * sw_i) * hh   for these two heads
                    x_ap = psc[:, :].rearrange("p (h j) -> p h j", h=2)[:, :, ai * P:(ai + 1) * P]  # [128, 2, 128]
                    hh_b = hh_a[:, :].unsqueeze(1).broadcast_to([P, 2, P])
                    nc.vector.scalar_tensor_tensor(
                        _r3(v1[:, hp * 2 * P:(hp + 1) * 2 * P]),
                        x_ap, swiT_s[:, a:a + 1], hh_b,
                        op0=OP.mult, op1=OP.mult)
                # --- V2 = V1 + (20*sw_i)*sw_j: scalar part on DVE (2x), add on GPSIMD
                w20 = stat_pool.tile([P, P], F32, tag="w20")
                nc.vector.tensor_scalar(
                    w20[:, :], swjb[:, ai * P:(ai + 1) * P], swi20T_s[:, a:a + 1], None,
                    op0=OP.mult)
                v2 = work_pool.tile([P, 4 * P], F32, tag="v2")
                w20_b = w20[:, :].unsqueeze(1).broadcast_to([P, NH, P])
                nc.gpsimd.tensor_tensor(
                    _r3(v2[:, :]), _r3(v1[:, :]), w20_b, op=OP.add)
                # --- E = exp(V2 - 60), fused row sums
                e_t = work_pool.tile([P, 4 * P], F32, tag="e")
                rows = stat_pool.tile([P, 3 * NH], F32, tag="rows")
                for h in range(NH):
                    nc.scalar.activation(
                        e_t[:, h * P:(h + 1) * P], v2[:, h * P:(h + 1) * P],
                        AF.Exp, bias=negb[:, 0:1], scale=1.0,
                        accum_out=rows[:, h:h + 1])
                nc.vector.reciprocal(rows[:, NH:2 * NH], rows[:, 0:NH])
                nc.vector.tensor_scalar(
                    rows[:, 2 * NH:], rows[:, NH:2 * NH], rmT_s[:, a:a + 1], None,
                    op0=OP.mult)
                # --- out_h = (E_h * rinv'_h) * hhm, interleaved write [i, j*4+h]
                ti = work_pool.tile([P, 4 * P], F32, tag="ti")
                ti3 = ti[:, :].rearrange("p (j h) -> p j h", h=NH)
                for h in range(NH):
                    nc.vector.scalar_tensor_tensor(
                        ti3[:, :, h], e_t[:, h * P:(h + 1) * P],
                        rows[:, 2 * NH + h:2 * NH + h + 1], hm_a[:, :],
                        op0=OP.mult, op1=OP.mult)
                nc.gpsimd.dma_start(out[a, :, :], ti[:, :])
    if not nc.is_finalized():
        nc.finalize()
    return nc


def _tf32(x):
    """Round-to-nearest float32 -> tf32 (10 mantissa bits), matching the PE's
    fp32r operand rounding so host-side values equal what the HW computes on."""
    u = np.ascontiguousarray(x, np.float32).view(np.uint32)
    r = (u + 0x1000 + ((u >> 13) & 1)) & 0xFFFFE000
    return r.view(np.float32)


def _host_prep(g2, h2, nlist_mask, sw, Wqk):
    """Build per-core input maps (host-side numpy prep)."""
    nb, nloc, nnei, din = g2.shape
    ATOT = nb * nloc
    A = ATOT // NCORES
    g2f = np.ascontiguousarray(g2.reshape(ATOT, nnei, din), np.float32)
    h2f = h2.reshape(ATOT, nnei, 3).astype(np.float32)
    maskf = nlist_mask.reshape(ATOT, nnei)
    swf = sw.reshape(ATOT, nnei).astype(np.float32)

    msw = swf * maskf  # [ATOT, 128]
    h2Tf = _tf32(np.ascontiguousarray(h2f.transpose(0, 2, 1)))          # [ATOT, 3, 128]
    h2swTf = _tf32(np.ascontiguousarray((h2f * swf[:, :, None]).transpose(0, 2, 1)))
    h2mTf = _tf32(np.ascontiguousarray((h2f * msw[:, :, None]).transpose(0, 2, 1)))
    g2Tf = _tf32(np.ascontiguousarray(g2f.transpose(1, 0, 2)))          # [128, ATOT, 64]

    # W2 per head: Wqk columns c = d*8 + h; q heads h<4, k heads h>=4
    Wqk64 = Wqk.astype(np.float64).reshape(din, ND, 2 * NH)
    w2p = np.zeros((din, NH * ND), np.float32)
    for h in range(NH):
        Wq = Wqk64[:, :, h]          # [64, 64]
        Wk = Wqk64[:, :, NH + h]
        W2 = (Wq @ Wk.T) / np.sqrt(np.float64(ND))
        w2p[:, h * ND:(h + 1) * ND] = W2.astype(np.float32)
    w2p = _tf32(w2p)

    ident = np.eye(P, dtype=np.float32)
    PACKW = A * DIN + 3 * A + P + 2 * P

    in_maps = []
    for c in range(NCORES):
        s = slice(c * A, (c + 1) * A)
        pk = np.zeros((P, PACKW), np.float32)
        o = A * DIN
        pk[:, 0:o] = g2Tf[:, s].reshape(P, A * DIN)
        pk[:, o:o + A] = swf[s].T; o += A
        pk[:, o:o + A] = (SHIFT * swf[s]).T; o += A
        pk[:, o:o + A] = (msw[s] / np.sqrt(np.float32(3.0))).T; o += A
        pk[:, o:o + P] = ident; o += P
        pk[0:DIN, o:o + 2 * P] = w2p; o += 2 * P
        in_maps.append({
            "pack": pk,
            "h2T": h2Tf[s],
            "h2swT": h2swTf[s],
            "h2mT": h2mTf[s],
            "swrow": np.ascontiguousarray(swf[s].reshape(1, A * P)),
        })
    return in_maps, A


_NC_CACHE = {}


def kernel(g2, h2, nlist_mask, sw, Wqk, _trace=False, _trace_kwargs=None):
    nb, nloc, nnei, din = g2.shape
    in_maps, A = _host_prep(g2, h2, nlist_mask, sw, Wqk)
    key = (A, str(MMDT))
    if key not in _NC_CACHE:
        _NC_CACHE[key] = build_nc(A)
    nc = _NC_CACHE[key]
    kw = {}
    if _trace:
        kw = dict(trace=True, **(_trace_kwargs or {}))
    res = run_bass_kernel_spmd(nc, in_maps, list(range(NCORES)), **kw)
    outs = [res.results[c]["out"] for c in range(NCORES)]
    full = np.concatenate(outs, axis=0)  # [1024, 128, 512]
    out = full.reshape(nb, nloc, nnei, nnei, NH).astype(np.float32)
    if _trace:
        return out, res
    return out


if __name__ == "__main__":
    # quick smoke test with tiny A on one build (still 8 cores)
    import reference as R
    inputs = {k: np.asarray(v) for k, v in R.setup_inputs().items()}
    out = kernel(**inputs)
    import jax.numpy as jnp
    ref = np.asarray(R.reference(**{k: jnp.asarray(v) for k, v in inputs.items()}))
    err = np.abs(out - ref)
    scale = np.abs(ref).max()
    print("absmax err:", err.max(), "scale:", scale, "scale-rel:", err.max() / scale)
    print("rel L2:", np.linalg.norm(err) / np.linalg.norm(ref))
